# revision 15
# baseline (speedup 1.0000x reference)
"""AWQ 4-bit quantized linear (x @ dequant(qweight).T + bias) on 8 Trainium2 cores.

Column-parallel sharding: out_features (O=11008, padded to 11264) split across
8 cores; x is replicated (fed transposed as xT so the contraction dim lands on
SBUF partitions). Each core dequantizes its weight shard on-device into SBUF
([I, O_sh] fp16, ~11.5 MB resident) and streams x tiles through the PE.

  kernel(x, qweight, qzeros, scales, bias) -> [8192, 11008] fp16
"""

import numpy as np
from contextlib import ExitStack

import concourse.bacc as bacc
import concourse.mybir as mybir
import concourse.tile as tile
from concourse._compat import with_exitstack
from concourse.bass_utils import run_bass_kernel_spmd


class _Bacc(bacc.Bacc):
    """Bacc that keeps matmuls self-loading.

    The stock `move_matmul_waits_to_ldweights` pass splits every InstMatmult
    into an explicit InstLdweights + InstMatmult; explicit LDWEIGHTS skips
    walrus's fast-weight-load codegen and measured ~117ns per matmul (~45ns
    un-hidden PE stall each). Self-loading matmuls let walrus emit the
    optimized weight load. Extra semaphore waits that the pass would have
    parked on the LDWEIGHTS are handled by `generate_event_semaphores`.
    """

    def move_matmul_waits_to_ldweights(self):
        pass

PACK = 8      # int32 packs 8 x 4-bit values, low nibble first
QBIT = 4
GS = 128      # quant group size == matmul k-tile size
NCORES = 8
TCH = 256     # t-columns fetched per x-tile DMA (2 PSUM t-tiles)

f16 = mybir.dt.float16
i16 = mybir.dt.int16
i32 = mybir.dt.int32
f32 = mybir.dt.float32
LSR = mybir.AluOpType.logical_shift_right
AND = mybir.AluOpType.bitwise_and
SUB = mybir.AluOpType.subtract
MUL = mybir.AluOpType.mult
ADD = mybir.AluOpType.add


def _n_splits(o_sh):
    # first split smallest so the first accumulation chains unlock after the
    # fewest dequantized o-tiles
    splits, off = [], 0
    rem = o_sh
    while rem > 0:
        n = rem % 512 if rem % 512 else 512
        splits.append((off, n))
        off += n
        rem -= n
    return splits


@with_exitstack
def _emit(ctx, tc, T, I, O_SH, xT, qw, qz, sc, b, out):
    nc = tc.nc
    KT = I // 128          # k-tiles (== quant groups, since GS == 128)
    NG = I // GS
    OT = O_SH // 128       # o-tiles per shard
    assert I % (128 * PACK) == 0 and T % TCH == 0 and O_SH % 128 == 0
    assert NG % PACK == 0

    const_pool = ctx.enter_context(tc.tile_pool(name="const", bufs=1))
    wt_pool = ctx.enter_context(tc.tile_pool(name="wt", bufs=1))
    deq_pool = ctx.enter_context(tc.tile_pool(name="deq", bufs=2))
    x_pool = ctx.enter_context(tc.tile_pool(name="x", bufs=2))
    o_pool = ctx.enter_context(tc.tile_pool(name="o", bufs=2))
    ps_pool = ctx.enter_context(tc.tile_pool(name="ps", bufs=2, space="PSUM"))

    bias_bc = const_pool.tile([128, O_SH], f16)
    nc.sync.dma_start(bias_bc[:], b.broadcast_to([128, O_SH]))

    # Resident dequantized, transposed weights: [128 (i in k-tile), KT, O_SH]
    WT = wt_pool.tile([128, KT, O_SH], f16)

    # ---- Phase A: dequantize the shard, o-tile by o-tile ----
    for j in range(OT):
        js = slice(j * 128, (j + 1) * 128)
        qwt = deq_pool.tile([128, I // PACK], i32, tag="qwt")
        nc.sync.dma_start(qwt[:], qw[js, :])
        qzt = deq_pool.tile([128, NG // PACK], i32, tag="qzt")
        nc.sync.dma_start(qzt[:], qz[js, :])
        sct = deq_pool.tile([128, NG], f16, tag="sct")
        nc.sync.dma_start(sct[:], sc[js, :])

        # bitvec ops can't cast, so unpack int32->int32 and cast in later ops
        zt_i = deq_pool.tile([128, NG], i32, tag="zt_i")
        uq = deq_pool.tile([128, I], i32, tag="uq")
        for k in range(PACK):
            nc.vector.tensor_scalar(zt_i[:, k::PACK], qzt[:], QBIT * k, 0xF, LSR, AND)
            nc.vector.tensor_scalar(uq[:, k::PACK], qwt[:], QBIT * k, 0xF, LSR, AND)
        zt = deq_pool.tile([128, NG], f32, tag="zt")
        nc.vector.tensor_copy(zt[:], zt_i[:])
        scf = deq_pool.tile([128, NG], f32, tag="scf")
        nc.vector.tensor_copy(scf[:], sct[:])
        # nzs = -z*s, so the dequant affine runs on the (otherwise idle)
        # scalar engine as activation: w = Copy(v*s + (-z*s))
        nzs = deq_pool.tile([128, NG], f32, tag="nzs")
        nc.vector.scalar_tensor_tensor(nzs[:], zt[:], -1.0, scf[:], MUL, MUL)

        # dequant affine, split 1/3 DVE : 2/3 ACT to balance the two engines'
        # phase-A load; transpose quarter-wise so each xbar DMA only waits on
        # 8 affines (fine-grained pipelining into WT)
        wq = deq_pool.tile([128, I], f16, tag="wq")
        gq = NG // 4
        for q in range(4):
            for gg in range(gq):
                g = q * gq + gg
                gs = slice(g * GS, (g + 1) * GS)
                if g % 3 == 0:
                    nc.vector.tensor_scalar(
                        wq[:, gs], uq[:, gs], zt[:, g : g + 1], scf[:, g : g + 1],
                        SUB, MUL,
                    )
                else:
                    nc.scalar.activation(
                        wq[:, gs], uq[:, gs], mybir.ActivationFunctionType.Identity,
                        bias=nzs[:, g : g + 1], scale=scf[:, g : g + 1],
                    )
            # WT[p, g, js+f] = wq[f, g*128+p] for g in this quarter
            nc.sync.dma_start_transpose(
                WT[:, q * gq : (q + 1) * gq, js],
                wq[:, q * gq * GS : (q + 1) * gq * GS],
            )

    # ---- Phase B: stream x through the PE ----
    splits = _n_splits(O_SH)
    xT_r = xT.rearrange("(k p) t -> p k t", p=128)  # [128, KT, T]
    for ti in range(T // TCH):
        xt = x_pool.tile([128, KT, TCH], f16, tag="xt")
        nc.sync.dma_start(xt[:], xT_r[:, :, ti * TCH : (ti + 1) * TCH])
        for h in range(TCH // 128):
            tsl = slice(h * 128, (h + 1) * 128)
            psums = [
                ps_pool.tile([128, nsz], f32, tag=f"ps{noff}", name=f"ps{noff}")
                for noff, nsz in splits
            ]
            for k in range(KT):
                for ps, (noff, nsz) in zip(psums, splits):
                    nc.tensor.matmul(
                        ps[:],
                        xt[:, k, tsl],
                        WT[:, k, noff : noff + nsz],
                        start=(k == 0),
                        stop=(k == KT - 1),
                    )
            # per-n-chain epilogue + store, so each PSUM slot recycles as soon
            # as its own chain finishes (no coupling across the 3 chains)
            t0 = ti * TCH + h * 128
            for ps, (noff, nsz) in zip(psums, splits):
                ot = o_pool.tile([128, nsz], f16, tag=f"ot{noff}", name=f"ot{noff}")
                nc.vector.tensor_tensor(
                    ot[:], ps[:], bias_bc[:, noff : noff + nsz], ADD
                )
                nc.sync.dma_start(out[t0 : t0 + 128, noff : noff + nsz], ot[:])


def _build(T, I, O_SH):
    nc = _Bacc(
        "TRN2",
        target_bir_lowering=False,
        debug=False,
        enable_asserts=False,
        num_devices=NCORES,
    )
    xT_d = nc.dram_tensor("xT", [I, T], f16, kind="ExternalInput")
    qw_d = nc.dram_tensor("qw", [O_SH, I // PACK], i32, kind="ExternalInput")
    qz_d = nc.dram_tensor("qz", [O_SH, I // GS // PACK], i32, kind="ExternalInput")
    sc_d = nc.dram_tensor("sc", [O_SH, I // GS], f16, kind="ExternalInput")
    b_d = nc.dram_tensor("b", [1, O_SH], f16, kind="ExternalInput")
    out_d = nc.dram_tensor("out", [T, O_SH], f16, kind="ExternalOutput")
    with tile.TileContext(nc) as tc:
        _emit(
            tc, T, I, O_SH,
            xT_d.ap(), qw_d.ap(), qz_d.ap(), sc_d.ap(), b_d.ap(), out_d.ap(),
        )
    nc.compile()
    return nc


_NC_CACHE = {}


def _get_nc(T, I, O_SH):
    key = (T, I, O_SH)
    if key not in _NC_CACHE:
        _NC_CACHE[key] = _build(*key)
    return _NC_CACHE[key]


def _shard_inputs(x, qweight, qzeros, scales, bias):
    T, I = x.shape
    O = qweight.shape[0]
    o_pad = -(-O // (128 * NCORES)) * (128 * NCORES)
    o_sh = o_pad // NCORES
    xT = np.ascontiguousarray(x.T)

    def pad_rows(a):
        if a.shape[0] == o_pad:
            return a
        pad = np.zeros((o_pad - a.shape[0],) + a.shape[1:], a.dtype)
        return np.concatenate([a, pad], axis=0)

    qw_p = pad_rows(np.asarray(qweight))
    qz_p = pad_rows(np.asarray(qzeros))
    sc_p = pad_rows(np.asarray(scales))
    b_p = pad_rows(np.asarray(bias))
    in_maps = []
    for c in range(NCORES):
        rows = slice(c * o_sh, (c + 1) * o_sh)
        in_maps.append(
            {
                "xT": xT,
                "qw": np.ascontiguousarray(qw_p[rows]),
                "qz": np.ascontiguousarray(qz_p[rows]),
                "sc": np.ascontiguousarray(sc_p[rows]),
                "b": np.ascontiguousarray(b_p[rows]).reshape(1, o_sh),
            }
        )
    return in_maps, T, I, O, o_sh


def _run(x, qweight, qzeros, scales, bias, trace=False, **kw):
    in_maps, T, I, O, o_sh = _shard_inputs(x, qweight, qzeros, scales, bias)
    nc = _get_nc(T, I, o_sh)
    res = run_bass_kernel_spmd(nc, in_maps, list(range(NCORES)), trace=trace, **kw)
    out = np.concatenate([res.results[c]["out"] for c in range(NCORES)], axis=1)
    return out[:, :O], res


def kernel(x, qweight, qzeros, scales, bias):
    out, _ = _run(x, qweight, qzeros, scales, bias)
    return out
